# revision 3
# baseline (speedup 1.0000x reference)
"""FourDirGradientConv + 1x1 compress + BatchNorm, Trainium2 Bass kernel (v3).

Two-stage formulation:
  stage 1 (PE): z_g = W_g @ x  (1x1 compress 32ch -> 16 z-channels, one
    block-diagonal lhsT reused by all 128 matmuls; K = 4 rows x 32 ch,
    M = 64 = 4 rows x 16 zch, out partition base alternates 0/64 so a
    PSUM bank pairs two 4-row blocks into the stage-2 K layout).
  stage 2 (PE): y = sum_g shift_g(z_g) - sum_g z_g  (3x3 stencil as
    matmuls over z with EXACT +-1 selector weights; K = 128 = 8 rows x
    16 zch, col shifts via rhs free offset on a padded z, row shifts in
    the selector planes; 16 matmuls per 32-row tile incl. edge taps).
  BN: per-tile bn_stats, [4,2] AllReduce across 8 cores, affine + store.

PE work: 128 + 252 = 380 matmuls of N=512 (vs 448 in v2).
z eviction PSUM->SBUF f16 alternates DVE/ACT, hidden under PE.

Sharding: data-parallel over batch, core b <-> sample b.
"""

import os
import numpy as np

import concourse.bass as bass
import concourse.tile as tile
import concourse.mybir as mybir
from concourse.bass_utils import run_bass_kernel_spmd


def _install_ntff_hook_shim():
    """Best-effort: provide antenv.axon_hooks (absent on this image) so the
    trace=True path of run_bass_kernel_spmd works if the caller enables it."""
    import sys, types
    if "antenv.axon_hooks" in sys.modules:
        return
    try:
        mod = types.ModuleType("antenv.axon_hooks")
        _state = {"hook": None}
        mod.set_axon_ntff_profile_hook = lambda h: _state.__setitem__("hook", h)
        mod.get_axon_ntff_profile_hook = lambda: _state["hook"]
        try:
            from trn_agent_boot.trn_boot import _ntff_profile_via_ctypes
            mod.set_axon_ntff_profile_hook(
                _ntff_profile_via_ctypes("/opt/axon/libaxon_pjrt.so")
            )
        except Exception:
            pass
        sys.modules["antenv.axon_hooks"] = mod
        import antenv
        antenv.axon_hooks = mod
    except Exception:
        pass


_install_ntff_hook_shim()

# problem constants (hardcoded per harness contract)
B, C, H, W = 8, 32, 512, 512
BN_EPS = 1e-5
N_CORES = 8

NB4 = H // 4     # 128 four-row blocks (stage-1 units)
NB8 = H // 8     # 64 eight-row blocks (stage-2 K units)
TILES = H // 32  # 16 output tiles of 32 rows
ZSLOTS = 32      # rolling z window (8-row blocks), >= 16 + slack
XSLOTS = 64      # rolling x window (4-row blocks)
WP = W + 2       # z padded width

F32 = mybir.dt.float32
F16 = mybir.dt.float16

# taps: group -> (di, dj); ne, nw, se, sw per reference order
TAPS = [(-1, 1), (-1, -1), (1, 1), (1, -1)]

# stage-2 selector planes: (j, dj) -> plane index; j in -1..4
PLANES = []
for _j in range(-1, 5):
    for _dj in (-1, 0, 1):
        if _j in (-1, 4) and _dj == 0:
            continue
        # skip planes that would be entirely empty
        PLANES.append((_j, _dj))
PLANE_IDX = {jd: i for i, jd in enumerate(PLANES)}
NPLANES = len(PLANES)  # 16

# x DMA chunk sizes over the 128 four-row blocks: fine early, coarse later
CHUNK_SIZES = [2, 2, 4, 4, 4, 4, 4] + [8] * 13


def _split_multiwait(nc, max_waits=1):
    """Walrus rejects >1 sync wait per instruction; hoist extras onto
    same-engine NOPs placed just before."""
    for f in nc.m.functions:
        for b in f.blocks:
            insts = list(b.instructions)
            out = []
            changed = False
            for inst in insts:
                si = inst.sync_info
                if si is not None and len(si.on_wait) > max_waits:
                    waits = list(si.on_wait)
                    keep = waits[-max_waits:]
                    for k, wt in enumerate(waits[:-max_waits]):
                        out.append(
                            mybir.InstNoOp(
                                name=f"{inst.name}-waitsplit-{k}",
                                engine=inst.engine,
                                sync_info=mybir.SyncInfo(on_wait=[wt], on_update=[]),
                            )
                        )
                    inst.sync_info = mybir.SyncInfo(
                        on_wait=keep, on_update=list(si.on_update)
                    )
                    changed = True
                out.append(inst)
            if changed:
                b.instructions = out


def build_module():
    nc = bass.Bass(num_devices=N_CORES)

    # xq[q4, ch, b4, c] = x[ch, 4*b4+q4, c], f16
    xb = nc.declare_dram_parameter("xq", [4, C, NB4, W], F16, isOutput=False)
    w1 = nc.declare_dram_parameter("w1", [128, 64], F16, isOutput=False)
    w2 = nc.declare_dram_parameter("w2", [128, NPLANES, 128], F16, isOutput=False)
    sel = nc.declare_dram_parameter("sel", [128, 4], F32, isOutput=False)
    selbc = nc.declare_dram_parameter("selbc", [4, 128], F32, isOutput=False)
    gamma = nc.declare_dram_parameter("gamma", [4, 1], F32, isOutput=False)
    beta = nc.declare_dram_parameter("beta", [4, 1], F32, isOutput=False)
    # y laid out as [rr, o, t, c] = out[o, 32t+rr, c]; host transposes back
    y = nc.declare_dram_parameter("y", [32, 4, TILES, W], F16, isOutput=True)

    with tile.TileContext(nc, num_cores=N_CORES) as tc:
        with (
            tc.tile_pool(name="xp", bufs=1) as xp,
            tc.tile_pool(name="zp", bufs=1) as zp,
            tc.tile_pool(name="const", bufs=1) as constp,
            tc.tile_pool(name="ysb", bufs=1) as ysbp,
            tc.tile_pool(name="stats", bufs=1) as statsp,
            tc.tile_pool(name="small", bufs=1) as smallp,
            tc.tile_pool(name="zps", bufs=3, space="PSUM") as zpsp,
            tc.tile_pool(name="yps", bufs=3, space="PSUM") as ypsp,
            tc.tile_pool(name="pss", bufs=1, space="PSUM") as pssp,
            tc.tile_pool(name="dram", bufs=1, space="DRAM") as dramp,
        ):
            x_t = xp.tile([128, NB4, W], F16)
            z_t = zp.tile([128, ZSLOTS, WP], F16)
            nc.gpsimd.memset(z_t[:, :, 0:1], 0.0)
            nc.gpsimd.memset(z_t[:, :, WP - 1 : WP], 0.0)

            w1_sb = constp.tile([128, 64], F16)
            w2_sb = constp.tile([128, NPLANES, 128], F16)
            sel_sb = constp.tile([128, 4], F32)
            selbc_sb = constp.tile([4, 128], F32)
            gamma_sb = constp.tile([4, 1], F32)
            beta_sb = constp.tile([4, 1], F32)
            eps_sb = constp.tile([4, 1], F32)
            nc.gpsimd.memset(eps_sb[:], BN_EPS)

            # weights ride the scalar queue ONLY (so ScalarE evictions are
            # never stuck behind bulk x traffic); x alternates sync/gpsimd
            nc.scalar.dma_start(out=w1_sb[:], in_=w1[:])
            nc.scalar.dma_start(out=w2_sb[:], in_=w2[:])
            b0 = 0
            for k, csz in enumerate(CHUNK_SIZES):
                dst = x_t[:, b0 : b0 + csz, :]
                nc.sync.dma_start(out=dst, in_=xb[:, :, b0 : b0 + csz, :])
                b0 += csz
            assert b0 == NB4
            nc.scalar.dma_start(out=sel_sb[:], in_=sel[:])
            nc.scalar.dma_start(out=selbc_sb[:], in_=selbc[:])
            nc.scalar.dma_start(out=gamma_sb[:], in_=gamma[:])
            nc.scalar.dma_start(out=beta_sb[:], in_=beta[:])

            y_sb = ysbp.tile([128, TILES, W], F16)
            stats = statsp.tile([128, TILES, 6], F32)

            # dummy collective: warms the gpsimd CC dispatch path so the real
            # AllReduce's trigger latency shrinks; result never read
            warm_in = dramp.tile([4, 2], F32)
            warm_out = dramp.tile([4, 2], F32)
            nc.gpsimd.collective_compute(
                "AllReduce",
                mybir.AluOpType.add,
                replica_groups=[list(range(N_CORES))],
                ins=[warm_in.opt()],
                outs=[warm_out.opt()],
            )

            # ---- main pipeline ----
            def emit_pair(m):
                """stage-1: 8-row z block m from 4-row x blocks 2m, 2m+1."""
                ps = zpsp.tile([128, W], F32, name="zps")
                nc.tensor.matmul(
                    out=ps[0:64, :], lhsT=w1_sb[:], rhs=x_t[:, 2 * m, :],
                    start=True, stop=True,
                )
                nc.tensor.matmul(
                    out=ps[64:128, :], lhsT=w1_sb[:], rhs=x_t[:, 2 * m + 1, :],
                    start=True, stop=True,
                )
                # both engines evict one half each: halves the latency that
                # gates the zps bank rotation for pair m+3
                h = W // 2
                nc.vector.tensor_copy(
                    out=z_t[:, m % ZSLOTS, 1 : 1 + h], in_=ps[:, 0:h]
                )
                nc.scalar.copy(
                    out=z_t[:, m % ZSLOTS, 1 + h : 1 + W], in_=ps[:, h:W]
                )

            def emit_tile(t):
                """stage-2: 32-row y tile t from z blocks 4t-1 .. 4t+4."""
                ps = ypsp.tile([128, W], F32, name="yps")
                mms = []
                for b in range(4 * t - 1, 4 * t + 5):
                    if b < 0 or b >= NB8:
                        continue
                    j = b - 4 * t
                    for dj in (-1, 0, 1):
                        if j in (-1, 4) and dj == 0:
                            continue
                        mms.append((b, j, dj))
                for n, (b, j, dj) in enumerate(mms):
                    nc.tensor.matmul(
                        out=ps[:],
                        lhsT=w2_sb[:, PLANE_IDX[(j, dj)], :],
                        rhs=z_t[:, b % ZSLOTS, 1 + dj : 1 + dj + W],
                        start=(n == 0),
                        stop=(n == len(mms) - 1),
                    )
                nc.vector.bn_stats(out=stats[:, t, :], in_=ps[:])
                nc.scalar.copy(out=y_sb[:, t, :], in_=ps[:])

            # ---- main pipeline: burst pairs, then tiles with lookahead;
            # warmup collective fires mid-loop so the CC path is hot ----
            for m in range(12):
                emit_pair(m)
            next_pair = 12
            for t in range(TILES):
                emit_tile(t)
                if t == 10:
                    nc.gpsimd.collective_compute(
                        "AllReduce",
                        mybir.AluOpType.add,
                        replica_groups=[list(range(N_CORES))],
                        ins=[warm_in.opt()],
                        outs=[warm_out.opt()],
                    )
                want = 4 * (t + 1) + 12
                while next_pair <= want and next_pair < NB8:
                    emit_pair(next_pair)
                    next_pair += 1
            assert next_pair == NB8

            # ---- BN stats combine + single AllReduce ----
            mv = smallp.tile([128, 2], F32)
            nc.vector.bn_aggr(out=mv[:], in_=stats[:])
            s12 = smallp.tile([128, 2], F32)
            nc.vector.tensor_copy(out=s12[:, 0:1], in_=mv[:, 0:1])
            nc.vector.tensor_tensor(
                out=s12[:, 1:2], in0=mv[:, 0:1], in1=mv[:, 0:1],
                op=mybir.AluOpType.mult,
            )
            nc.vector.tensor_tensor(
                out=s12[:, 1:2], in0=s12[:, 1:2], in1=mv[:, 1:2],
                op=mybir.AluOpType.add,
            )
            pss_t = pssp.tile([128, 2], F32)
            nc.tensor.matmul(
                out=pss_t[0:4, :], lhsT=sel_sb[:], rhs=s12[:],
                start=True, stop=True,
            )
            comb = smallp.tile([4, 2], F32)
            nc.scalar.copy(out=comb[:], in_=pss_t[0:4, :])
            cc_in = dramp.tile([4, 2], F32)
            cc_out = dramp.tile([4, 2], F32)
            nc.sync.dma_start(out=cc_in[:], in_=comb[:])
            nc.gpsimd.collective_compute(
                "AllReduce",
                mybir.AluOpType.add,
                replica_groups=[list(range(N_CORES))],
                ins=[cc_in.opt()],
                outs=[cc_out.opt()],
            )
            arin = smallp.tile([4, 2], F32)
            nc.sync.dma_start(out=arin[:], in_=cc_out[:])

            mean = arin[:, 0:1]
            var = smallp.tile([4, 1], F32)
            nc.vector.tensor_copy(out=var[:], in_=arin[:, 1:2])
            msq = smallp.tile([4, 1], F32)
            nc.vector.tensor_tensor(
                out=msq[:], in0=mean, in1=mean, op=mybir.AluOpType.mult
            )
            nc.vector.tensor_tensor(
                out=var[:], in0=var[:], in1=msq[:], op=mybir.AluOpType.subtract
            )
            sd = smallp.tile([4, 1], F32)
            nc.scalar.activation(
                out=sd[:], in_=var[:], func=mybir.ActivationFunctionType.Sqrt,
                bias=eps_sb[:], scale=1.0,
            )
            rstd = smallp.tile([4, 1], F32)
            nc.vector.reciprocal(out=rstd[:], in_=sd[:])
            scbi = smallp.tile([4, 2], F32)
            nc.vector.tensor_tensor(
                out=scbi[:, 0:1], in0=gamma_sb[:], in1=rstd[:],
                op=mybir.AluOpType.mult,
            )
            tmp = smallp.tile([4, 1], F32)
            nc.vector.tensor_tensor(
                out=tmp[:], in0=mean, in1=scbi[:, 0:1], op=mybir.AluOpType.mult
            )
            nc.vector.tensor_tensor(
                out=scbi[:, 1:2], in0=beta_sb[:], in1=tmp[:],
                op=mybir.AluOpType.subtract,
            )
            nc.tensor.matmul(
                out=pss_t[:], lhsT=selbc_sb[:], rhs=scbi[:], start=True, stop=True
            )
            scv = smallp.tile([128, 2], F32)
            nc.scalar.copy(out=scv[:], in_=pss_t[:])

            # ---- affine + store: vector and scalar engines in parallel,
            # graduated chunks (big first, tiny last) ----
            STORE_PLAN = [
                ("v", 3), ("s", 2), ("v", 3), ("s", 2),
                ("v", 2), ("s", 1), ("v", 2), ("v", 1),
            ]
            t0c = 0
            for h, (engk, ssz) in enumerate(STORE_PLAN):
                sl = slice(t0c, t0c + ssz)
                t0c += ssz
                if engk == "v":
                    nc.vector.tensor_scalar(
                        out=y_sb[:, sl, :],
                        in0=y_sb[:, sl, :],
                        scalar1=scv[:, 0:1],
                        scalar2=scv[:, 1:2],
                        op0=mybir.AluOpType.mult,
                        op1=mybir.AluOpType.add,
                    )
                    nc.sync.dma_start(out=y[:, :, sl, :], in_=y_sb[:, sl, :])
                else:
                    nc.scalar.activation(
                        out=y_sb[:, sl, :],
                        in_=y_sb[:, sl, :],
                        func=mybir.ActivationFunctionType.Identity,
                        bias=scv[:, 1:2],
                        scale=scv[:, 0:1],
                    )
                    nc.scalar.dma_start(out=y[:, :, sl, :], in_=y_sb[:, sl, :])
            assert t0c == TILES

    _split_multiwait(nc)
    return nc


def _host_constants(w_compress):
    # Wfull[oc = 4g+o, ch] = w_compress[o, 32g + ch]
    wfull = np.zeros((16, 32), dtype=np.float32)
    for g in range(4):
        for o in range(4):
            wfull[4 * g + o] = w_compress[o, 32 * g : 32 * g + 32]

    # stage-1 lhsT: w1[32*q4 + ch, 16*q4 + oc] = Wfull[oc, ch]
    w1 = np.zeros((128, 64), dtype=np.float32)
    for q4 in range(4):
        w1[32 * q4 : 32 * q4 + 32, 16 * q4 : 16 * q4 + 16] = wfull.T

    # stage-2 selector planes: w2[plane, 16*q8 + 4g + o, 4*rr + o] = +-1
    w2 = np.zeros((NPLANES, 128, 128), dtype=np.float32)
    for (j, dj), idx in PLANE_IDX.items():
        for q8 in range(8):
            for g, (di, djg) in enumerate(TAPS):
                for o in range(4):
                    if dj == 0:
                        rr = 8 * j + q8
                        if 0 <= rr < 32:
                            w2[idx, 16 * q8 + 4 * g + o, 4 * rr + o] += -1.0
                    elif djg == dj:
                        rr = 8 * j + q8 - di
                        if 0 <= rr < 32:
                            w2[idx, 16 * q8 + 4 * g + o, 4 * rr + o] += 1.0

    sel = np.zeros((128, 4), dtype=np.float32)
    for prt in range(128):
        sel[prt, prt % 4] = 1.0 / 256.0
    selbc = np.zeros((4, 128), dtype=np.float32)
    for prt in range(128):
        selbc[prt % 4, prt] = 1.0
    return w1, w2, sel, selbc


_NC_CACHE = {}


def kernel(x, w_compress, gamma, beta):
    x = np.ascontiguousarray(np.asarray(x, dtype=np.float32))
    w_compress = np.asarray(w_compress, dtype=np.float32)
    gamma = np.asarray(gamma, dtype=np.float32)
    beta = np.asarray(beta, dtype=np.float32)

    if "nc" not in _NC_CACHE:
        _NC_CACHE["nc"] = build_module()
    nc = _NC_CACHE["nc"]

    w1, w2, sel, selbc = _host_constants(w_compress)
    w1 = w1.astype(np.float16)
    w2 = np.ascontiguousarray(w2.transpose(1, 0, 2)).astype(np.float16)
    in_maps = []
    for b in range(B):
        xq = np.ascontiguousarray(
            x[b].reshape(C, NB4, 4, W).transpose(2, 0, 1, 3).astype(np.float16)
        )
        in_maps.append(
            {
                "xq": xq,
                "w1": w1,
                "w2": w2,
                "sel": sel,
                "selbc": selbc,
                "gamma": gamma.reshape(4, 1),
                "beta": beta.reshape(4, 1),
            }
        )
    res = run_bass_kernel_spmd(
        nc,
        in_maps,
        core_ids=list(range(N_CORES)),
        trace=os.environ.get("BASSK_TRACE", "0") == "1",
    )
    _NC_CACHE["last_result"] = res
    out = np.stack(
        [
            res.results[b]["y"].astype(np.float32).transpose(1, 2, 0, 3).reshape(4, H, W)
            for b in range(B)
        ],
        axis=0,
    )
    return out


# revision 4
# speedup vs baseline: 1.0267x; 1.0267x over previous
"""FourDirGradientConv + 1x1 compress + BatchNorm, Trainium2 Bass kernel (v3).

Two-stage formulation:
  stage 1 (PE): z_g = W_g @ x  (1x1 compress 32ch -> 16 z-channels, one
    block-diagonal lhsT reused by all 128 matmuls; K = 4 rows x 32 ch,
    M = 64 = 4 rows x 16 zch, out partition base alternates 0/64 so a
    PSUM bank pairs two 4-row blocks into the stage-2 K layout).
  stage 2 (PE): y = sum_g shift_g(z_g) - sum_g z_g  (3x3 stencil as
    matmuls over z with EXACT +-1 selector weights; K = 128 = 8 rows x
    16 zch, col shifts via rhs free offset on a padded z, row shifts in
    the selector planes; 16 matmuls per 32-row tile incl. edge taps).
  BN: per-tile bn_stats, [4,2] AllReduce across 8 cores, affine + store.

PE work: 128 + 252 = 380 matmuls of N=512 (vs 448 in v2).
z eviction PSUM->SBUF f16 alternates DVE/ACT, hidden under PE.

Sharding: data-parallel over batch, core b <-> sample b.
"""

import os
import numpy as np

import concourse.bass as bass
import concourse.tile as tile
import concourse.mybir as mybir
from concourse.bass_utils import run_bass_kernel_spmd


def _install_ntff_hook_shim():
    """Best-effort: provide antenv.axon_hooks (absent on this image) so the
    trace=True path of run_bass_kernel_spmd works if the caller enables it."""
    import sys, types
    if "antenv.axon_hooks" in sys.modules:
        return
    try:
        mod = types.ModuleType("antenv.axon_hooks")
        _state = {"hook": None}
        mod.set_axon_ntff_profile_hook = lambda h: _state.__setitem__("hook", h)
        mod.get_axon_ntff_profile_hook = lambda: _state["hook"]
        try:
            from trn_agent_boot.trn_boot import _ntff_profile_via_ctypes
            mod.set_axon_ntff_profile_hook(
                _ntff_profile_via_ctypes("/opt/axon/libaxon_pjrt.so")
            )
        except Exception:
            pass
        sys.modules["antenv.axon_hooks"] = mod
        import antenv
        antenv.axon_hooks = mod
    except Exception:
        pass


_install_ntff_hook_shim()

# problem constants (hardcoded per harness contract)
B, C, H, W = 8, 32, 512, 512
BN_EPS = 1e-5
N_CORES = 8

NB4 = H // 4     # 128 four-row blocks (stage-1 units)
NB8 = H // 8     # 64 eight-row blocks (stage-2 K units)
TILES = H // 32  # 16 output tiles of 32 rows
ZSLOTS = 32      # rolling z window (8-row blocks), >= 16 + slack
XSLOTS = 64      # rolling x window (4-row blocks)
WP = W + 2       # z padded width

F32 = mybir.dt.float32
F16 = mybir.dt.float16

# taps: group -> (di, dj); ne, nw, se, sw per reference order
TAPS = [(-1, 1), (-1, -1), (1, 1), (1, -1)]

# stage-2 selector planes: (j, dj) -> plane index; j in -1..4
PLANES = []
for _j in range(-1, 5):
    for _dj in (-1, 0, 1):
        if _j in (-1, 4) and _dj == 0:
            continue
        # skip planes that would be entirely empty
        PLANES.append((_j, _dj))
PLANE_IDX = {jd: i for i, jd in enumerate(PLANES)}
NPLANES = len(PLANES)  # 16

# x DMA chunk sizes over the 128 four-row blocks: fine early, coarse later
CHUNK_SIZES = [2, 2, 4, 4, 4, 4, 4] + [8] * 13


def _split_multiwait(nc, max_waits=1):
    """Walrus rejects >1 sync wait per instruction; hoist extras onto
    same-engine NOPs placed just before."""
    for f in nc.m.functions:
        for b in f.blocks:
            insts = list(b.instructions)
            out = []
            changed = False
            for inst in insts:
                si = inst.sync_info
                if si is not None and len(si.on_wait) > max_waits:
                    waits = list(si.on_wait)
                    keep = waits[-max_waits:]
                    for k, wt in enumerate(waits[:-max_waits]):
                        out.append(
                            mybir.InstNoOp(
                                name=f"{inst.name}-waitsplit-{k}",
                                engine=inst.engine,
                                sync_info=mybir.SyncInfo(on_wait=[wt], on_update=[]),
                            )
                        )
                    inst.sync_info = mybir.SyncInfo(
                        on_wait=keep, on_update=list(si.on_update)
                    )
                    changed = True
                out.append(inst)
            if changed:
                b.instructions = out


def build_module():
    nc = bass.Bass(num_devices=N_CORES)

    # xq[q4, ch, b4, c] = x[ch, 4*b4+q4, c], f16
    xb = nc.declare_dram_parameter("xq", [4, C, NB4, W], F16, isOutput=False)
    w1 = nc.declare_dram_parameter("w1", [128, 64], F16, isOutput=False)
    w2 = nc.declare_dram_parameter("w2", [128, NPLANES, 128], F16, isOutput=False)
    sel = nc.declare_dram_parameter("sel", [128, 4], F32, isOutput=False)
    selbc = nc.declare_dram_parameter("selbc", [4, 128], F32, isOutput=False)
    gamma = nc.declare_dram_parameter("gamma", [4, 1], F32, isOutput=False)
    beta = nc.declare_dram_parameter("beta", [4, 1], F32, isOutput=False)
    # y laid out as [rr, o, t, c] = out[o, 32t+rr, c]; host transposes back
    y = nc.declare_dram_parameter("y", [32, 4, TILES, W], F16, isOutput=True)

    with tile.TileContext(nc, num_cores=N_CORES) as tc:
        with (
            tc.tile_pool(name="xp", bufs=1) as xp,
            tc.tile_pool(name="zp", bufs=1) as zp,
            tc.tile_pool(name="const", bufs=1) as constp,
            tc.tile_pool(name="ysb", bufs=1) as ysbp,
            tc.tile_pool(name="stats", bufs=1) as statsp,
            tc.tile_pool(name="small", bufs=1) as smallp,
            tc.tile_pool(name="zps", bufs=3, space="PSUM") as zpsp,
            tc.tile_pool(name="yps", bufs=3, space="PSUM") as ypsp,
            tc.tile_pool(name="pss", bufs=1, space="PSUM") as pssp,
            tc.tile_pool(name="dram", bufs=1, space="DRAM") as dramp,
        ):
            x_t = xp.tile([128, NB4, W], F16)
            z_t = zp.tile([128, ZSLOTS, WP], F16)
            nc.gpsimd.memset(z_t[:, :, 0:1], 0.0)
            nc.gpsimd.memset(z_t[:, :, WP - 1 : WP], 0.0)

            w1_sb = constp.tile([128, 64], F16)
            w2_sb = constp.tile([128, NPLANES, 128], F16)
            sel_sb = constp.tile([128, 4], F32)
            selbc_sb = constp.tile([4, 128], F32)
            gamma_sb = constp.tile([4, 1], F32)
            beta_sb = constp.tile([4, 1], F32)
            eps_sb = constp.tile([4, 1], F32)
            nc.gpsimd.memset(eps_sb[:], BN_EPS)

            # weights ride the scalar queue ONLY (so ScalarE evictions are
            # never stuck behind bulk x traffic); x alternates sync/gpsimd
            nc.scalar.dma_start(out=w1_sb[:], in_=w1[:])
            nc.scalar.dma_start(out=w2_sb[:], in_=w2[:])
            b0 = 0
            for k, csz in enumerate(CHUNK_SIZES):
                dst = x_t[:, b0 : b0 + csz, :]
                nc.sync.dma_start(out=dst, in_=xb[:, :, b0 : b0 + csz, :])
                b0 += csz
            assert b0 == NB4
            nc.scalar.dma_start(out=sel_sb[:], in_=sel[:])
            nc.scalar.dma_start(out=selbc_sb[:], in_=selbc[:])
            nc.scalar.dma_start(out=gamma_sb[:], in_=gamma[:])
            nc.scalar.dma_start(out=beta_sb[:], in_=beta[:])

            y_sb = ysbp.tile([128, TILES, W], F16)
            stats = statsp.tile([128, TILES, 6], F32)

            # dummy collective: warms the gpsimd CC dispatch path so the real
            # AllReduce's trigger latency shrinks; result never read
            warm_in = dramp.tile([4, 2], F32)
            warm_out = dramp.tile([4, 2], F32)
            nc.gpsimd.collective_compute(
                "AllReduce",
                mybir.AluOpType.add,
                replica_groups=[list(range(N_CORES))],
                ins=[warm_in.opt()],
                outs=[warm_out.opt()],
            )

            # ---- main pipeline ----
            def emit_pair(m):
                """stage-1: 8-row z block m from 4-row x blocks 2m, 2m+1."""
                ps = zpsp.tile([128, W], F32, name="zps")
                nc.tensor.matmul(
                    out=ps[0:64, :], lhsT=w1_sb[:], rhs=x_t[:, 2 * m, :],
                    start=True, stop=True,
                )
                nc.tensor.matmul(
                    out=ps[64:128, :], lhsT=w1_sb[:], rhs=x_t[:, 2 * m + 1, :],
                    start=True, stop=True,
                )
                # both engines evict one half each: halves the latency that
                # gates the zps bank rotation for pair m+3
                h = W // 2
                nc.vector.tensor_copy(
                    out=z_t[:, m % ZSLOTS, 1 : 1 + h], in_=ps[:, 0:h]
                )
                nc.scalar.copy(
                    out=z_t[:, m % ZSLOTS, 1 + h : 1 + W], in_=ps[:, h:W]
                )

            def emit_tile(t, due_pairs):
                """stage-2: 32-row y tile t from z blocks 4t-1 .. 4t+4.
                due_pairs (future stage-1 blocks, never read by this tile)
                are interleaved into the mm stream so evictions get slack
                before their zps bank is reused -- keeps the PE gap-free."""
                ps = ypsp.tile([128, W], F32, name="yps")
                mms = []
                for b in range(4 * t - 1, 4 * t + 5):
                    if b < 0 or b >= NB8:
                        continue
                    j = b - 4 * t
                    for dj in (-1, 0, 1):
                        if j in (-1, 4) and dj == 0:
                            continue
                        mms.append((b, j, dj))
                pi = 0
                for n, (b, j, dj) in enumerate(mms):
                    nc.tensor.matmul(
                        out=ps[:],
                        lhsT=w2_sb[:, PLANE_IDX[(j, dj)], :],
                        rhs=z_t[:, b % ZSLOTS, 1 + dj : 1 + dj + W],
                        start=(n == 0),
                        stop=(n == len(mms) - 1),
                    )
                    if n % 4 == 3 and pi < len(due_pairs):
                        emit_pair(due_pairs[pi])
                        pi += 1
                for m in due_pairs[pi:]:
                    emit_pair(m)
                nc.vector.bn_stats(out=stats[:, t, :], in_=ps[:])
                nc.scalar.copy(out=y_sb[:, t, :], in_=ps[:])

            # ---- main pipeline: burst pairs, then tiles with lookahead;
            # warmup collective fires mid-loop so the CC path is hot ----
            for m in range(12):
                emit_pair(m)
            next_pair = 12
            for t in range(TILES):
                want = 4 * (t + 1) + 12
                due = list(range(next_pair, min(want + 1, NB8)))
                if due:
                    next_pair = due[-1] + 1
                emit_tile(t, due)
                if t == 10:
                    nc.gpsimd.collective_compute(
                        "AllReduce",
                        mybir.AluOpType.add,
                        replica_groups=[list(range(N_CORES))],
                        ins=[warm_in.opt()],
                        outs=[warm_out.opt()],
                    )
            assert next_pair == NB8

            # ---- BN stats combine + single AllReduce ----
            mv = smallp.tile([128, 2], F32)
            nc.vector.bn_aggr(out=mv[:], in_=stats[:])
            s12 = smallp.tile([128, 2], F32)
            nc.vector.tensor_copy(out=s12[:, 0:1], in_=mv[:, 0:1])
            nc.vector.tensor_tensor(
                out=s12[:, 1:2], in0=mv[:, 0:1], in1=mv[:, 0:1],
                op=mybir.AluOpType.mult,
            )
            nc.vector.tensor_tensor(
                out=s12[:, 1:2], in0=s12[:, 1:2], in1=mv[:, 1:2],
                op=mybir.AluOpType.add,
            )
            pss_t = pssp.tile([128, 2], F32)
            nc.tensor.matmul(
                out=pss_t[0:4, :], lhsT=sel_sb[:], rhs=s12[:],
                start=True, stop=True,
            )
            comb = smallp.tile([4, 2], F32)
            nc.scalar.copy(out=comb[:], in_=pss_t[0:4, :])
            cc_in = dramp.tile([4, 2], F32)
            cc_out = dramp.tile([4, 2], F32)
            nc.sync.dma_start(out=cc_in[:], in_=comb[:])
            nc.gpsimd.collective_compute(
                "AllReduce",
                mybir.AluOpType.add,
                replica_groups=[list(range(N_CORES))],
                ins=[cc_in.opt()],
                outs=[cc_out.opt()],
            )
            arin = smallp.tile([4, 2], F32)
            nc.sync.dma_start(out=arin[:], in_=cc_out[:])

            mean = arin[:, 0:1]
            var = smallp.tile([4, 1], F32)
            nc.vector.tensor_copy(out=var[:], in_=arin[:, 1:2])
            msq = smallp.tile([4, 1], F32)
            nc.vector.tensor_tensor(
                out=msq[:], in0=mean, in1=mean, op=mybir.AluOpType.mult
            )
            nc.vector.tensor_tensor(
                out=var[:], in0=var[:], in1=msq[:], op=mybir.AluOpType.subtract
            )
            sd = smallp.tile([4, 1], F32)
            nc.scalar.activation(
                out=sd[:], in_=var[:], func=mybir.ActivationFunctionType.Sqrt,
                bias=eps_sb[:], scale=1.0,
            )
            rstd = smallp.tile([4, 1], F32)
            nc.vector.reciprocal(out=rstd[:], in_=sd[:])
            scbi = smallp.tile([4, 2], F32)
            nc.vector.tensor_tensor(
                out=scbi[:, 0:1], in0=gamma_sb[:], in1=rstd[:],
                op=mybir.AluOpType.mult,
            )
            tmp = smallp.tile([4, 1], F32)
            nc.vector.tensor_tensor(
                out=tmp[:], in0=mean, in1=scbi[:, 0:1], op=mybir.AluOpType.mult
            )
            nc.vector.tensor_tensor(
                out=scbi[:, 1:2], in0=beta_sb[:], in1=tmp[:],
                op=mybir.AluOpType.subtract,
            )
            nc.tensor.matmul(
                out=pss_t[:], lhsT=selbc_sb[:], rhs=scbi[:], start=True, stop=True
            )
            scv = smallp.tile([128, 2], F32)
            nc.scalar.copy(out=scv[:], in_=pss_t[:])

            # ---- affine + store: vector and scalar engines in parallel,
            # graduated chunks (big first, tiny last) ----
            STORE_PLAN = [
                ("v", 3), ("s", 2), ("v", 3), ("s", 2),
                ("v", 2), ("s", 1), ("v", 2), ("v", 1),
            ]
            t0c = 0
            for h, (engk, ssz) in enumerate(STORE_PLAN):
                sl = slice(t0c, t0c + ssz)
                t0c += ssz
                if engk == "v":
                    nc.vector.tensor_scalar(
                        out=y_sb[:, sl, :],
                        in0=y_sb[:, sl, :],
                        scalar1=scv[:, 0:1],
                        scalar2=scv[:, 1:2],
                        op0=mybir.AluOpType.mult,
                        op1=mybir.AluOpType.add,
                    )
                    nc.sync.dma_start(out=y[:, :, sl, :], in_=y_sb[:, sl, :])
                else:
                    nc.scalar.activation(
                        out=y_sb[:, sl, :],
                        in_=y_sb[:, sl, :],
                        func=mybir.ActivationFunctionType.Identity,
                        bias=scv[:, 1:2],
                        scale=scv[:, 0:1],
                    )
                    nc.scalar.dma_start(out=y[:, :, sl, :], in_=y_sb[:, sl, :])
            assert t0c == TILES

    _split_multiwait(nc)
    return nc


def _host_constants(w_compress):
    # Wfull[oc = 4g+o, ch] = w_compress[o, 32g + ch]
    wfull = np.zeros((16, 32), dtype=np.float32)
    for g in range(4):
        for o in range(4):
            wfull[4 * g + o] = w_compress[o, 32 * g : 32 * g + 32]

    # stage-1 lhsT: w1[32*q4 + ch, 16*q4 + oc] = Wfull[oc, ch]
    w1 = np.zeros((128, 64), dtype=np.float32)
    for q4 in range(4):
        w1[32 * q4 : 32 * q4 + 32, 16 * q4 : 16 * q4 + 16] = wfull.T

    # stage-2 selector planes: w2[plane, 16*q8 + 4g + o, 4*rr + o] = +-1
    w2 = np.zeros((NPLANES, 128, 128), dtype=np.float32)
    for (j, dj), idx in PLANE_IDX.items():
        for q8 in range(8):
            for g, (di, djg) in enumerate(TAPS):
                for o in range(4):
                    if dj == 0:
                        rr = 8 * j + q8
                        if 0 <= rr < 32:
                            w2[idx, 16 * q8 + 4 * g + o, 4 * rr + o] += -1.0
                    elif djg == dj:
                        rr = 8 * j + q8 - di
                        if 0 <= rr < 32:
                            w2[idx, 16 * q8 + 4 * g + o, 4 * rr + o] += 1.0

    sel = np.zeros((128, 4), dtype=np.float32)
    for prt in range(128):
        sel[prt, prt % 4] = 1.0 / 256.0
    selbc = np.zeros((4, 128), dtype=np.float32)
    for prt in range(128):
        selbc[prt % 4, prt] = 1.0
    return w1, w2, sel, selbc


_NC_CACHE = {}


def kernel(x, w_compress, gamma, beta):
    x = np.ascontiguousarray(np.asarray(x, dtype=np.float32))
    w_compress = np.asarray(w_compress, dtype=np.float32)
    gamma = np.asarray(gamma, dtype=np.float32)
    beta = np.asarray(beta, dtype=np.float32)

    if "nc" not in _NC_CACHE:
        _NC_CACHE["nc"] = build_module()
    nc = _NC_CACHE["nc"]

    w1, w2, sel, selbc = _host_constants(w_compress)
    w1 = w1.astype(np.float16)
    w2 = np.ascontiguousarray(w2.transpose(1, 0, 2)).astype(np.float16)
    in_maps = []
    for b in range(B):
        xq = np.ascontiguousarray(
            x[b].reshape(C, NB4, 4, W).transpose(2, 0, 1, 3).astype(np.float16)
        )
        in_maps.append(
            {
                "xq": xq,
                "w1": w1,
                "w2": w2,
                "sel": sel,
                "selbc": selbc,
                "gamma": gamma.reshape(4, 1),
                "beta": beta.reshape(4, 1),
            }
        )
    res = run_bass_kernel_spmd(
        nc,
        in_maps,
        core_ids=list(range(N_CORES)),
        trace=os.environ.get("BASSK_TRACE", "0") == "1",
    )
    _NC_CACHE["last_result"] = res
    out = np.stack(
        [
            res.results[b]["y"].astype(np.float32).transpose(1, 2, 0, 3).reshape(4, H, W)
            for b in range(B)
        ],
        axis=0,
    )
    return out
